# revision 1
# baseline (speedup 1.0000x reference)
"""Trainium2 Bass kernel for ConvolutionalAttention (B=2,S=2048,E=1024,H=16,KS=3).

Reference:  Q,K,V = query @ W.T + b;  scores = QK^T/sqrt(Dh) per head;
cross-head conv1d (H->H channels, kernel 3) along the key axis; softmax over
keys; out = (weights @ V) merged heads @ Wo.T + bo.

Strategy (8 cores, head-parallel, conv folded into K):
  K_conv[ho][k,(hi,d)] = sum_dk conv_w[ho,hi,dk] * K[k+dk-1,(hi,d)]
  => scores_conv[ho] = Q_full @ K_conv[ho]^T   (E=1024-deep matmul, computed
  transposed as [k,q]).  Each core owns H/8 = 2 output heads for all (b,q):
    1. one pass over host-transposed query^T computes Q^T (->DRAM, bf16),
       K^T (->SBUF, zero-padded edge cols) and V[s,d] (->SBUF), sharing every
       loaded rhs tile between the three projections;
    2. K_conv formed on VectorE (3 shifted, per-partition-scaled taps)->DRAM;
    3. per (b, head): QK_conv matmuls (bf16) -> PSUM f32 -> Exp on ScalarE
       (bf16 out) -> PV matmuls against ones-augmented V so the softmax
       denominator lands in PSUM row 64 -> reciprocal -> K=1-matmul broadcast
       -> normalize (+bv).  bv is exact post-softmax (weights sum to 1);
       conv_b cancels inside softmax; 1/sqrt(Dh) folded into Wq/bq on host;
    4. AllToAll reshards (head-slice -> q-slice); final Wo projection of this
       core's 512 output rows (f32r matmuls).
"""
import numpy as np
import ml_dtypes

import concourse.bacc as bacc
import concourse.mybir as mybir
import concourse.tile as tile
from concourse.bass_utils import run_bass_kernel_spmd

B, S, E, H, KS = 2, 2048, 1024, 16, 3
DH = E // H                  # 64
N_CORES = 8
HPC = H // N_CORES           # 2 heads per core
BS = B * S                   # 4096
QSLICE = BS // N_CORES       # 512 output rows per core
NE = E // 128                # 8 contraction chunks
NSC = BS // 512              # 8 s-chunks in projection pass
NKT = S // 128               # 16 k-tiles per batch
NQC = S // 512               # 4 q-chunks per batch
VROW = DH + 1                # 65: head block in augmented V
KT_PAD_W = 2 * S + 4         # [z | b0:S | z z | b1:S | z]
_B_OFF = (1, S + 3)
_PAD_COLS = (0, S + 1, S + 2, 2 * S + 3)

F32 = mybir.dt.float32
F32R = mybir.dt.float32r
BF16 = mybir.dt.bfloat16
AL = mybir.AluOpType
AF = mybir.ActivationFunctionType


def _r(ap):
    return ap.bitcast(F32R)


def build_nc(n_cores=N_CORES, collective=True):
    nc = bacc.Bacc("TRN2", target_bir_lowering=False, debug=False,
                   num_devices=n_cores)
    # inputs (host-prepped layouts; see prep_in_maps)
    qTh = nc.dram_tensor("qTh", [E, BS], BF16, kind="ExternalInput")
    wq_p = nc.dram_tensor("wq_p", [128, NE * NE * 128], BF16, kind="ExternalInput")
    wk_p = nc.dram_tensor("wk_p", [128, NE * NE * 128], BF16, kind="ExternalInput")
    wv_p = nc.dram_tensor("wv_p", [128, NE * HPC * DH], BF16, kind="ExternalInput")
    wo_p = nc.dram_tensor("wo_p", [128, NE * E], F32R, kind="ExternalInput")
    bq = nc.dram_tensor("bq", [128, NE], F32, kind="ExternalInput")
    bk = nc.dram_tensor("bk", [128, NE], F32, kind="ExternalInput")
    bv = nc.dram_tensor("bv", [128, HPC], F32, kind="ExternalInput")
    bo = nc.dram_tensor("bo", [128, E], F32, kind="ExternalInput")
    wvec = nc.dram_tensor("wvec", [128, HPC * KS * NE], F32, kind="ExternalInput")
    out = nc.dram_tensor("out", [QSLICE, E], F32, kind="ExternalOutput")

    with tile.TileContext(nc) as tc:
        with (
            tc.tile_pool(name="dram", bufs=1, space="DRAM") as dram,
            tc.tile_pool(name="persist", bufs=1) as persist,
        ):
            qproj_dram = dram.tile([E, BS], BF16)
            kconv_dram = dram.tile([HPC, E, BS], BF16)
            a2a_in = dram.tile([N_CORES * 128, QSLICE], F32)
            a2a_out = dram.tile([N_CORES * 128, QSLICE], F32)

            # augmented V: cols = g*(HPC*VROW) + h*VROW + [0..63]=d, 64=ones
            # where g = b*NKT + kt is the global k-tile index (32 of them)
            v_sb = persist.tile([128, B * NKT * HPC * VROW], BF16)
            bv_sb = persist.tile([128, HPC], F32)
            wvec_sb = persist.tile([128, HPC * KS * NE], F32)
            ones_sb = persist.tile([1, DH], BF16)
            nc.sync.dma_start(bv_sb[:], bv[:, :])
            nc.sync.dma_start(wvec_sb[:], wvec[:, :])
            nc.vector.memset(ones_sb[:], 1.0)
            for g in range(B * NKT):
                for h in range(HPC):
                    c0 = g * HPC * VROW + h * VROW + DH
                    nc.vector.memset(v_sb[:, c0:c0 + 1], 1.0)

            # ---------------- phase 1: projections ----------------
            with (
                tc.tile_pool(name="proj", bufs=1) as proj,
                tc.tile_pool(name="pw", bufs=2) as pw,
                tc.tile_pool(name="pevac", bufs=3) as pevac,
                tc.tile_pool(name="ppsum", bufs=2, space="PSUM") as ppsum,
                tc.tile_pool(name="vpsum", bufs=4, space="PSUM") as vpsum,
            ):
                qt_full = proj.tile([128, NE * BS], BF16, tag="qtfull")
                kt_pad = proj.tile([128, NE * KT_PAD_W], BF16, tag="ktpad")
                wv_sb = proj.tile([128, NE * HPC * DH], BF16, tag="wv")
                bq_sb = proj.tile([128, NE], F32, tag="bq")
                bk_sb = proj.tile([128, NE], F32, tag="bk")
                for j in range(NE):
                    nc.sync.dma_start(qt_full[:, j * BS:(j + 1) * BS],
                                      qTh[j * 128:(j + 1) * 128, :])
                nc.sync.dma_start(wv_sb[:], wv_p[:, :])
                nc.sync.dma_start(bq_sb[:], bq[:, :])
                nc.sync.dma_start(bk_sb[:], bk[:, :])
                for c in range(NE):
                    for pc in _PAD_COLS:
                        col = c * KT_PAD_W + pc
                        nc.vector.memset(kt_pad[:, col:col + 1], 0.0)

                # Q^T and K^T: for each e-tile stream the packed weight stripe
                for et in range(NE):
                    wq_sb = pw.tile([128, NE * 128], BF16, tag="wqs")
                    wk_sb = pw.tile([128, NE * 128], BF16, tag="wks")
                    nc.sync.dma_start(wq_sb[:], wq_p[:, et * E:(et + 1) * E])
                    nc.sync.dma_start(wk_sb[:], wk_p[:, et * E:(et + 1) * E])
                    for sc in range(NSC):
                        b_i, sc_i = divmod(sc, NQC)
                        pq = ppsum.tile([128, 512], F32, tag="pq")
                        pk = ppsum.tile([128, 512], F32, tag="pk")
                        for j in range(NE):
                            rhs = qt_full[:, j * BS + sc * 512:j * BS + (sc + 1) * 512]
                            nc.tensor.matmul(pq[:], wq_sb[:, j * 128:(j + 1) * 128],
                                             rhs, start=(j == 0), stop=(j == NE - 1))
                        for j in range(NE):
                            rhs = qt_full[:, j * BS + sc * 512:j * BS + (sc + 1) * 512]
                            nc.tensor.matmul(pk[:], wk_sb[:, j * 128:(j + 1) * 128],
                                             rhs, start=(j == 0), stop=(j == NE - 1))
                        qe = pevac.tile([128, 512], BF16, tag="qevac")
                        nc.scalar.activation(qe[:], pq[:], AF.Identity,
                                             bias=bq_sb[:, et:et + 1], scale=1.0)
                        nc.sync.dma_start(
                            qproj_dram[et * 128:(et + 1) * 128,
                                       sc * 512:(sc + 1) * 512], qe[:])
                        kcol = et * KT_PAD_W + _B_OFF[b_i] + sc_i * 512
                        nc.scalar.activation(kt_pad[:, kcol:kcol + 512], pk[:],
                                             AF.Identity,
                                             bias=bk_sb[:, et:et + 1], scale=1.0)

                # V: lhsT = raw query^T tiles, rhs = packed Wv^T slice
                for g in range(B * NKT):           # g = s-tile = global k-tile
                    pv = vpsum.tile([128, HPC * DH], F32, tag="pv")
                    for j in range(NE):
                        lhsT = qt_full[:, j * BS + g * 128:j * BS + (g + 1) * 128]
                        nc.tensor.matmul(
                            pv[:], lhsT,
                            wv_sb[:, j * HPC * DH:(j + 1) * HPC * DH],
                            start=(j == 0), stop=(j == NE - 1))
                    for h in range(HPC):
                        c0 = g * HPC * VROW + h * VROW
                        nc.scalar.activation(v_sb[:, c0:c0 + DH],
                                             pv[:, h * DH:(h + 1) * DH], AF.Copy)

                # ---------------- phase 2: K_conv on VectorE ----------------
                with tc.tile_pool(name="kcv", bufs=2) as kcv:
                    for h in range(HPC):
                        for c in range(NE):
                            for b_i in range(B):
                                base = c * KT_PAD_W + _B_OFF[b_i] - 1
                                t0 = kcv.tile([128, S], BF16, tag="t0")
                                t1 = kcv.tile([128, S], BF16, tag="t1")

                                def wv_(dk):
                                    col = (h * KS + dk) * NE + c
                                    return wvec_sb[:, col:col + 1]

                                nc.vector.tensor_scalar(
                                    t0[:], kt_pad[:, base:base + S],
                                    wv_(0), None, AL.mult)
                                nc.vector.tensor_scalar(
                                    t1[:], kt_pad[:, base + 1:base + 1 + S],
                                    wv_(1), None, AL.mult)
                                nc.vector.tensor_tensor(t0[:], t0[:], t1[:], AL.add)
                                nc.vector.tensor_scalar(
                                    t1[:], kt_pad[:, base + 2:base + 2 + S],
                                    wv_(2), None, AL.mult)
                                nc.vector.tensor_tensor(t0[:], t0[:], t1[:], AL.add)
                                nc.sync.dma_start(
                                    kconv_dram[h, c * 128:(c + 1) * 128,
                                               b_i * S:(b_i + 1) * S], t0[:])

            # ---------------- phase 3: attention ----------------
            with (
                tc.tile_pool(name="attn", bufs=1) as attn,
                tc.tile_pool(name="kcs", bufs=2) as kcs,
                tc.tile_pool(name="esb", bufs=6) as esb,
                tc.tile_pool(name="norm", bufs=4) as norm,
                tc.tile_pool(name="qkpsum", bufs=2, space="PSUM") as qkpsum,
                tc.tile_pool(name="pvpsum", bufs=4, space="PSUM") as pvpsum,
                tc.tile_pool(name="ivpsum", bufs=2, space="PSUM") as ivpsum,
            ):
                for b_i in range(B):
                    qp_res = attn.tile([128, NE * S], BF16, tag="qpres")
                    for j in range(NE):
                        nc.sync.dma_start(
                            qp_res[:, j * S:(j + 1) * S],
                            qproj_dram[j * 128:(j + 1) * 128,
                                       b_i * S:(b_i + 1) * S])
                    for h in range(HPC):
                        pvs = [pvpsum.tile([VROW, 512], F32, tag="pvp",
                                           name=f"pv{qq}")
                               for qq in range(NQC)]
                        inv_sb = norm.tile([1, S], BF16, tag="inv")
                        for kg in range(NKT // 4):
                            kc_sbs = []
                            for j in range(NE):
                                kt_sb = kcs.tile([128, 512], BF16,
                                                 tag=f"kc{j}", name=f"kcs{j}")
                                nc.sync.dma_start(
                                    kt_sb[:],
                                    kconv_dram[h, j * 128:(j + 1) * 128,
                                               b_i * S + kg * 512:
                                               b_i * S + (kg + 1) * 512])
                                kc_sbs.append(kt_sb)
                            for t in range(4):
                                kt = kg * 4 + t
                                g = b_i * NKT + kt
                                for qc in range(NQC):
                                    ps = qkpsum.tile([128, 512], F32, tag="qk")
                                    for j in range(NE):
                                        nc.tensor.matmul(
                                            ps[:],
                                            kc_sbs[j][:, t * 128:(t + 1) * 128],
                                            qp_res[:, j * S + qc * 512:
                                                   j * S + (qc + 1) * 512],
                                            start=(j == 0), stop=(j == NE - 1))
                                    ex = esb.tile([128, 512], BF16, tag="exp")
                                    nc.scalar.activation(ex[:], ps[:], AF.Exp)
                                    c0 = g * HPC * VROW + h * VROW
                                    nc.tensor.matmul(
                                        pvs[qc][:], v_sb[:, c0:c0 + VROW],
                                        ex[:], start=(kt == 0),
                                        stop=(kt == NKT - 1))
                        # normalize + bias, ship to a2a bounce
                        for qc in range(NQC):
                            with nc.allow_low_precision(
                                    reason="softmax denom bf16 bcast"):
                                nc.vector.reciprocal(
                                    inv_sb[0:1, qc * 512:(qc + 1) * 512],
                                    pvs[qc][DH:DH + 1, :])
                            pi = ivpsum.tile([DH, 512], F32, tag="iv")
                            nc.tensor.matmul(
                                pi[:], ones_sb[0:1, :],
                                inv_sb[0:1, qc * 512:(qc + 1) * 512],
                                start=True, stop=True)
                            ib = norm.tile([DH, 512], F32, tag="invbc")
                            nc.scalar.activation(ib[:], pi[:], AF.Copy)
                            ho = norm.tile([DH, 512], F32, tag="ho")
                            nc.vector.tensor_tensor(ho[:], pvs[qc][0:DH, :],
                                                    ib[:], AL.mult)
                            nc.vector.tensor_scalar(
                                ho[:], ho[:], bv_sb[0:DH, h:h + 1], None, AL.add)
                            piece = b_i * NQC + qc
                            r0 = piece * 128 + h * DH
                            nc.sync.dma_start(a2a_in[r0:r0 + DH, :], ho[:])

            # ---------------- phase 4: exchange + output proj ----------------
            if collective:
                nc.gpsimd.collective_compute(
                    "AllToAll", AL.bypass,
                    replica_groups=[list(range(n_cores))],
                    ins=[a2a_in.opt()], outs=[a2a_out.opt()])
            else:
                nc.sync.dma_start(a2a_out[:, :], a2a_in[:, :])

            with (
                tc.tile_pool(name="fin", bufs=1) as fin,
                tc.tile_pool(name="fevac", bufs=3) as fevac,
                tc.tile_pool(name="fpsum", bufs=2, space="PSUM") as fpsum,
            ):
                go_sb = fin.tile([128, NE * QSLICE], F32R, tag="go")
                wo_sb = fin.tile([128, NE * E], F32R, tag="wo")
                bo_sb = fin.tile([128, E], F32, tag="bo")
                for e in range(NE):
                    nc.sync.dma_start(go_sb[:, e * QSLICE:(e + 1) * QSLICE],
                                      _r(a2a_out[e * 128:(e + 1) * 128, :]))
                nc.sync.dma_start(wo_sb[:], wo_p[:, :])
                nc.sync.dma_start(bo_sb[:], bo[:, :])
                for qt in range(QSLICE // 128):
                    for nh in range(E // 512):
                        pf = fpsum.tile([128, 512], F32, tag="pf")
                        for e in range(NE):
                            lhsT = go_sb[:, e * QSLICE + qt * 128:
                                         e * QSLICE + (qt + 1) * 128]
                            rhs = wo_sb[:, e * E + nh * 512:e * E + (nh + 1) * 512]
                            nc.tensor.matmul(pf[:], lhsT, rhs,
                                             start=(e == 0), stop=(e == NE - 1))
                        ot = fevac.tile([128, 512], F32, tag="ot")
                        nc.vector.tensor_tensor(
                            ot[:], pf[:], bo_sb[:, nh * 512:(nh + 1) * 512],
                            AL.add)
                        nc.sync.dma_start(
                            out[qt * 128:(qt + 1) * 128,
                                nh * 512:(nh + 1) * 512], ot[:])
    nc.compile()
    return nc


def prep_in_maps(query, Wq, bq, Wk, bk, Wv, bv, Wo, bo, conv_w, conv_b):
    """Host-side layout prep. conv_b is dropped: softmax(x+c) == softmax(x)."""
    del conv_b
    scale = 1.0 / np.sqrt(DH)
    qT = np.ascontiguousarray(query.reshape(BS, E).T)           # [E, BS]
    qTh = qT.astype(ml_dtypes.bfloat16)

    def pack_w(WT):  # [E_in, E_out] -> [128, NE*NE*128], stripe et is
        # [128, NE*128] with col (j*128+e) = WT[j*128+p, et*128+e]
        Wr = WT.reshape(NE, 128, NE, 128)          # [j, p, et, e]
        return np.ascontiguousarray(
            Wr.transpose(1, 2, 0, 3).reshape(128, NE * NE * 128))

    wq_p = pack_w((Wq.T * scale).astype(np.float32)).astype(ml_dtypes.bfloat16)
    wk_p = pack_w(Wk.T.astype(np.float32)).astype(ml_dtypes.bfloat16)
    # wo packed: stripe e is [128, E] with col eo = Wo.T[e*128+p, eo]
    wo_p = np.ascontiguousarray(
        Wo.T.reshape(NE, 128, E).transpose(1, 0, 2).reshape(128, NE * E)
    ).astype(np.float32)
    bq_a = np.ascontiguousarray((bq * scale).reshape(NE, 128).T).astype(np.float32)
    bk_a = np.ascontiguousarray(bk.reshape(NE, 128).T).astype(np.float32)
    bo_a = np.tile(bo.astype(np.float32)[None, :], (128, 1))

    in_maps = []
    for c in range(N_CORES):
        heads = [HPC * c + h for h in range(HPC)]
        # Wv^T slice packed: [128, NE*HPC*DH], col block j -> Wv[e_g, j*128+p]
        wv_cols = np.concatenate(
            [Wv[ho * DH:(ho + 1) * DH, :] for ho in heads], axis=0)  # [128,E]
        wv_p = np.ascontiguousarray(
            wv_cols.T.reshape(NE, 128, HPC * DH).transpose(1, 0, 2)
            .reshape(128, NE * HPC * DH)).astype(ml_dtypes.bfloat16)
        bv_a = np.zeros((128, HPC), np.float32)
        for h, ho in enumerate(heads):
            bv_a[0:DH, h] = bv[ho * DH:(ho + 1) * DH]
        wvec = np.zeros((128, HPC * KS * NE), np.float32)
        for h, ho in enumerate(heads):
            for dk in range(KS):
                col_v = np.repeat(conv_w[ho, :, dk], DH)       # [E]
                for cc in range(NE):
                    wvec[:, (h * KS + dk) * NE + cc] = col_v[cc * 128:(cc + 1) * 128]
        in_maps.append({
            "qTh": qTh, "wq_p": wq_p, "wk_p": wk_p, "wv_p": wv_p,
            "wo_p": wo_p, "bq": bq_a, "bk": bk_a, "bv": bv_a,
            "bo": bo_a, "wvec": wvec,
        })
    return in_maps


_NC_CACHE = {}


def kernel(**inputs) -> np.ndarray:
    in_maps = prep_in_maps(**{k: np.asarray(v) for k, v in inputs.items()})
    if "nc" not in _NC_CACHE:
        _NC_CACHE["nc"] = build_nc()
    nc = _NC_CACHE["nc"]
    res = run_bass_kernel_spmd(nc, in_maps, list(range(N_CORES)))
    full = np.concatenate([res.results[c]["out"] for c in range(N_CORES)],
                          axis=0)
    return full.reshape(B, S, E).astype(np.float32)



# revision 2
# speedup vs baseline: 107.3275x; 107.3275x over previous
"""Trainium2 Bass kernel for ConvolutionalAttention (B=2,S=2048,E=1024,H=16,KS=3).

Reference:  Q,K,V = query @ W.T + b;  scores = QK^T/sqrt(Dh) per head;
cross-head conv1d (H->H channels, kernel 3) along the key axis; softmax over
keys; out = (weights @ V) merged heads @ Wo.T + bo.

Strategy (8 cores, head-parallel, conv folded into K):
  K_conv[ho][k,(hi,d)] = sum_dk conv_w[ho,hi,dk] * K[k+dk-1,(hi,d)]
  => scores_conv[ho] = Q_full @ K_conv[ho]^T   (E=1024-deep matmul, computed
  transposed as [k,q]).  Each core owns H/8 = 2 output heads for all (b,q):
    1. one pass over host-transposed query^T computes Q^T (->DRAM, fp8 scaled
       by SQ), K^T (->SBUF bf16, zero-padded edge cols) and V[s,d] (->SBUF),
       sharing every loaded rhs tile between the three projections; K_conv
       for e-chunk c is emitted right after its projections (VectorE overlaps
       the next chunk's matmuls);
    2. K_conv formed on VectorE (3 shifted, per-partition-scaled taps, scaled
       by SK) -> DRAM fp8;
    3. per (b, head): QK_conv fp8 DoubleRow matmuls (256-deep each) -> PSUM
       f32 -> Exp on ScalarE with scale 1/(SQ*SK) (bf16 out) -> PV matmuls
       against ones-augmented V so the softmax denominator lands in PSUM row
       64 -> reciprocal -> K=1-matmul broadcast -> normalize (+bv).  bv is
       exact post-softmax (weights sum to 1); conv_b cancels inside softmax;
       1/sqrt(Dh) folded into Wq/bq on host;
    4. AllToAll (bf16) reshards (head-slice -> q-slice); final Wo projection
       of this core's 512 output rows (bf16 matmuls, f32 bias+output).
"""
import numpy as np
import ml_dtypes

import concourse.bacc as bacc
import concourse.mybir as mybir
import concourse.tile as tile
from concourse.bass_utils import run_bass_kernel_spmd

B, S, E, H, KS = 2, 2048, 1024, 16, 3
DH = E // H                  # 64
N_CORES = 8
HPC = H // N_CORES           # 2 heads per core
BS = B * S                   # 4096
QSLICE = BS // N_CORES       # 512 output rows per core
NE = E // 128                # 8 contraction chunks
NCC = NE // 2                # 4 DoubleRow (256-deep) chunks
NSC = BS // 512              # 8 s-chunks in projection pass
NKT = S // 128               # 16 k-tiles per batch
NQC = S // 512               # 4 q-chunks per batch
VROW = DH + 1                # 65: head block in augmented V
KT_PAD_W = 2 * S + 4         # [z | b0:S | z z | b1:S | z]
_B_OFF = (1, S + 3)
_PAD_COLS = (0, S + 1, S + 2, 2 * S + 3)

SQ = 256.0                   # fp8 scale on Q^T (exact power of 2)
SK = 256.0                   # fp8 scale on K_conv
EXP_SCALE = 1.0 / (SQ * SK)

F32 = mybir.dt.float32
BF16 = mybir.dt.bfloat16
F8 = mybir.dt.float8e4
AL = mybir.AluOpType
AF = mybir.ActivationFunctionType
DR = mybir.MatmulPerfMode.DoubleRow


def build_nc(n_cores=N_CORES, collective=True):
    nc = bacc.Bacc("TRN2", target_bir_lowering=False, debug=False,
                   num_devices=n_cores)
    # inputs (host-prepped layouts; see prep_in_maps)
    qTh = nc.dram_tensor("qTh", [E, BS], BF16, kind="ExternalInput")
    wq_p = nc.dram_tensor("wq_p", [128, NE * NE * 128], BF16, kind="ExternalInput")
    wk_p = nc.dram_tensor("wk_p", [128, NE * NE * 128], BF16, kind="ExternalInput")
    wv_p = nc.dram_tensor("wv_p", [128, NE * HPC * DH], BF16, kind="ExternalInput")
    wo_p = nc.dram_tensor("wo_p", [128, NE * E], BF16, kind="ExternalInput")
    bq = nc.dram_tensor("bq", [128, NE], F32, kind="ExternalInput")
    bk = nc.dram_tensor("bk", [128, NE], F32, kind="ExternalInput")
    bv = nc.dram_tensor("bv", [128, HPC], F32, kind="ExternalInput")
    bo = nc.dram_tensor("bo", [128, E], F32, kind="ExternalInput")
    wvec = nc.dram_tensor("wvec", [128, HPC * KS * NE], F32, kind="ExternalInput")
    out = nc.dram_tensor("out", [QSLICE, E], F32, kind="ExternalOutput")

    with tile.TileContext(nc) as tc:
        with (
            tc.tile_pool(name="dram", bufs=1, space="DRAM") as dram,
            tc.tile_pool(name="persist", bufs=1) as persist,
        ):
            qproj_dram = dram.tile([E, BS], F8)
            kconv_dram = dram.tile([HPC, E, BS], F8)
            a2a_in = dram.tile([N_CORES * 128, QSLICE], BF16)
            a2a_out = dram.tile([N_CORES * 128, QSLICE], BF16)

            # augmented V: cols = g*(HPC*VROW) + h*VROW + [0..63]=d, 64=ones
            # where g = b*NKT + kt is the global k-tile index (32 of them)
            v_sb = persist.tile([128, B * NKT * HPC * VROW], BF16)
            bv_sb = persist.tile([128, HPC], F32)
            wvec_sb = persist.tile([128, HPC * KS * NE], F32)
            ones_sb = persist.tile([1, DH], BF16)
            nc.sync.dma_start(bv_sb[:], bv[:, :])
            nc.sync.dma_start(wvec_sb[:], wvec[:, :])
            nc.vector.memset(ones_sb[:], 1.0)
            for g in range(B * NKT):
                for h in range(HPC):
                    c0 = g * HPC * VROW + h * VROW + DH
                    nc.vector.memset(v_sb[:, c0:c0 + 1], 1.0)

            # ------- phase 1: projections (+ K_conv interleaved) -------
            with (
                tc.tile_pool(name="proj", bufs=1) as proj,
                tc.tile_pool(name="pw", bufs=2) as pw,
                tc.tile_pool(name="pevac", bufs=3) as pevac,
                tc.tile_pool(name="kcv", bufs=2) as kcv,
                tc.tile_pool(name="ppsum", bufs=2, space="PSUM") as ppsum,
                tc.tile_pool(name="vpsum", bufs=4, space="PSUM") as vpsum,
            ):
                qt_full = proj.tile([128, NE * BS], BF16, tag="qtfull")
                kt_pad = proj.tile([128, NE * KT_PAD_W], BF16, tag="ktpad")
                wv_sb = proj.tile([128, NE * HPC * DH], BF16, tag="wv")
                bq_sb = proj.tile([128, NE], F32, tag="bq")
                bk_sb = proj.tile([128, NE], F32, tag="bk")
                for j in range(NE):
                    nc.sync.dma_start(qt_full[:, j * BS:(j + 1) * BS],
                                      qTh[j * 128:(j + 1) * 128, :])
                nc.sync.dma_start(wv_sb[:], wv_p[:, :])
                nc.sync.dma_start(bq_sb[:], bq[:, :])
                nc.sync.dma_start(bk_sb[:], bk[:, :])
                for c in range(NE):
                    for pc in _PAD_COLS:
                        col = c * KT_PAD_W + pc
                        nc.vector.memset(kt_pad[:, col:col + 1], 0.0)

                def emit_kconv(c):
                    # cross-head conv along k on VectorE for e-chunk c
                    for h in range(HPC):
                        for b_i in range(B):
                            base = c * KT_PAD_W + _B_OFF[b_i] - 1
                            t0 = kcv.tile([128, S], BF16, tag="t0")
                            t1 = kcv.tile([128, S], BF16, tag="t1")
                            t2 = kcv.tile([128, S], F8, tag="t2")

                            def wv_(dk):
                                col = (h * KS + dk) * NE + c
                                return wvec_sb[:, col:col + 1]

                            nc.vector.tensor_scalar(
                                t0[:], kt_pad[:, base:base + S],
                                wv_(0), None, AL.mult)
                            nc.vector.tensor_scalar(
                                t1[:], kt_pad[:, base + 1:base + 1 + S],
                                wv_(1), None, AL.mult)
                            nc.vector.tensor_tensor(t0[:], t0[:], t1[:], AL.add)
                            nc.vector.tensor_scalar(
                                t1[:], kt_pad[:, base + 2:base + 2 + S],
                                wv_(2), None, AL.mult)
                            nc.vector.tensor_tensor(t2[:], t0[:], t1[:], AL.add)
                            nc.sync.dma_start(
                                kconv_dram[h, c * 128:(c + 1) * 128,
                                           b_i * S:(b_i + 1) * S], t2[:])

                # Q^T and K^T: for each e-tile stream the packed weight stripe
                for et in range(NE):
                    wq_sb = pw.tile([128, NE * 128], BF16, tag="wqs")
                    wk_sb = pw.tile([128, NE * 128], BF16, tag="wks")
                    nc.sync.dma_start(wq_sb[:], wq_p[:, et * E:(et + 1) * E])
                    nc.sync.dma_start(wk_sb[:], wk_p[:, et * E:(et + 1) * E])
                    for sc in range(NSC):
                        b_i, sc_i = divmod(sc, NQC)
                        pq = ppsum.tile([128, 512], F32, tag="pq")
                        pk = ppsum.tile([128, 512], F32, tag="pk")
                        for j in range(NE):
                            rhs = qt_full[:, j * BS + sc * 512:j * BS + (sc + 1) * 512]
                            nc.tensor.matmul(pq[:], wq_sb[:, j * 128:(j + 1) * 128],
                                             rhs, start=(j == 0), stop=(j == NE - 1))
                        for j in range(NE):
                            rhs = qt_full[:, j * BS + sc * 512:j * BS + (sc + 1) * 512]
                            nc.tensor.matmul(pk[:], wk_sb[:, j * 128:(j + 1) * 128],
                                             rhs, start=(j == 0), stop=(j == NE - 1))
                        qe = pevac.tile([128, 512], F8, tag="qevac")
                        nc.scalar.activation(qe[:], pq[:], AF.Identity,
                                             bias=bq_sb[:, et:et + 1], scale=1.0)
                        nc.sync.dma_start(
                            qproj_dram[et * 128:(et + 1) * 128,
                                       sc * 512:(sc + 1) * 512], qe[:])
                        kcol = et * KT_PAD_W + _B_OFF[b_i] + sc_i * 512
                        nc.scalar.activation(kt_pad[:, kcol:kcol + 512], pk[:],
                                             AF.Identity,
                                             bias=bk_sb[:, et:et + 1], scale=1.0)
                    emit_kconv(et)

                # V: lhsT = raw query^T tiles, rhs = packed Wv^T slice
                for g in range(B * NKT):           # g = s-tile = global k-tile
                    pv = vpsum.tile([128, HPC * DH], F32, tag="pv")
                    for j in range(NE):
                        lhsT = qt_full[:, j * BS + g * 128:j * BS + (g + 1) * 128]
                        nc.tensor.matmul(
                            pv[:], lhsT,
                            wv_sb[:, j * HPC * DH:(j + 1) * HPC * DH],
                            start=(j == 0), stop=(j == NE - 1))
                    for h in range(HPC):
                        c0 = g * HPC * VROW + h * VROW
                        nc.scalar.activation(v_sb[:, c0:c0 + DH],
                                             pv[:, h * DH:(h + 1) * DH], AF.Copy)

            # ---------------- phase 3: attention ----------------
            with (
                tc.tile_pool(name="attn", bufs=1) as attn,
                tc.tile_pool(name="kcs", bufs=2) as kcs,
                tc.tile_pool(name="esb", bufs=6) as esb,
                tc.tile_pool(name="norm", bufs=4) as norm,
                tc.tile_pool(name="qkpsum", bufs=2, space="PSUM") as qkpsum,
                tc.tile_pool(name="pvpsum", bufs=4, space="PSUM") as pvpsum,
                tc.tile_pool(name="ivpsum", bufs=2, space="PSUM") as ivpsum,
            ):
                for b_i in range(B):
                    qp_res = attn.tile([128, NE, S], F8, tag="qpres")
                    for j in range(NE):
                        nc.sync.dma_start(
                            qp_res[:, j, :],
                            qproj_dram[j * 128:(j + 1) * 128,
                                       b_i * S:(b_i + 1) * S])
                    for h in range(HPC):
                        # whole-S K_conv slab for this (b,h): 4 DoubleRow pair
                        # tiles [128, 2, S] (pair cc covers e-rows cc*256 ..)
                        kc_sbs = []
                        for cc in range(NCC):
                            kt_sb = kcs.tile([128, 2, S], F8,
                                             tag=f"kc{cc}", name=f"kcs{cc}")
                            for i in range(2):
                                nc.sync.dma_start(
                                    kt_sb[:, i, :],
                                    kconv_dram[h,
                                               cc * 256 + i * 128:
                                               cc * 256 + (i + 1) * 128,
                                               b_i * S:(b_i + 1) * S])
                            kc_sbs.append(kt_sb)
                        pvs = [pvpsum.tile([VROW, 512], F32, tag="pvp",
                                           name=f"pv{qq}")
                               for qq in range(NQC)]
                        inv_sb = norm.tile([1, S], BF16, tag="inv")
                        for kt in range(NKT):
                            g = b_i * NKT + kt
                            for qc in range(NQC):
                                ps = qkpsum.tile([128, 512], F32, tag="qk")
                                for cc in range(NCC):
                                    nc.tensor.matmul(
                                        ps[:],
                                        kc_sbs[cc][:, :, kt * 128:(kt + 1) * 128],
                                        qp_res[:, 2 * cc:2 * cc + 2,
                                               qc * 512:(qc + 1) * 512],
                                        start=(cc == 0), stop=(cc == NCC - 1),
                                        perf_mode=DR)
                                ex = esb.tile([128, 512], BF16, tag="exp")
                                nc.scalar.activation(ex[:], ps[:], AF.Exp,
                                                     scale=EXP_SCALE)
                                c0 = g * HPC * VROW + h * VROW
                                nc.tensor.matmul(
                                    pvs[qc][:], v_sb[:, c0:c0 + VROW],
                                    ex[:], start=(kt == 0),
                                    stop=(kt == NKT - 1))
                        # normalize + bias, ship to a2a bounce
                        for qc in range(NQC):
                            with nc.allow_low_precision(
                                    reason="softmax denom bf16 bcast"):
                                nc.vector.reciprocal(
                                    inv_sb[0:1, qc * 512:(qc + 1) * 512],
                                    pvs[qc][DH:DH + 1, :])
                            pi = ivpsum.tile([DH, 512], F32, tag="iv")
                            nc.tensor.matmul(
                                pi[:], ones_sb[0:1, :],
                                inv_sb[0:1, qc * 512:(qc + 1) * 512],
                                start=True, stop=True)
                            ib = norm.tile([DH, 512], F32, tag="invbc")
                            nc.scalar.activation(ib[:], pi[:], AF.Copy)
                            ho = norm.tile([DH, 512], F32, tag="ho")
                            ho16 = norm.tile([DH, 512], BF16, tag="ho16")
                            nc.vector.tensor_tensor(ho[:], pvs[qc][0:DH, :],
                                                    ib[:], AL.mult)
                            nc.vector.tensor_scalar(
                                ho16[:], ho[:], bv_sb[0:DH, h:h + 1], None,
                                AL.add)
                            piece = b_i * NQC + qc
                            r0 = piece * 128 + h * DH
                            nc.sync.dma_start(a2a_in[r0:r0 + DH, :], ho16[:])

            # ---------------- phase 4: exchange + output proj ----------------
            if collective:
                nc.gpsimd.collective_compute(
                    "AllToAll", AL.bypass,
                    replica_groups=[list(range(n_cores))],
                    ins=[a2a_in.opt()], outs=[a2a_out.opt()])
            else:
                nc.sync.dma_start(a2a_out[:, :], a2a_in[:, :])

            with (
                tc.tile_pool(name="fin", bufs=1) as fin,
                tc.tile_pool(name="fevac", bufs=3) as fevac,
                tc.tile_pool(name="fpsum", bufs=2, space="PSUM") as fpsum,
            ):
                go_sb = fin.tile([128, NE * QSLICE], BF16, tag="go")
                wo_sb = fin.tile([128, NE * E], BF16, tag="wo")
                bo_sb = fin.tile([128, E], F32, tag="bo")
                for e in range(NE):
                    nc.sync.dma_start(go_sb[:, e * QSLICE:(e + 1) * QSLICE],
                                      a2a_out[e * 128:(e + 1) * 128, :])
                nc.sync.dma_start(wo_sb[:], wo_p[:, :])
                nc.sync.dma_start(bo_sb[:], bo[:, :])
                for qt in range(QSLICE // 128):
                    for nh in range(E // 512):
                        pf = fpsum.tile([128, 512], F32, tag="pf")
                        for e in range(NE):
                            lhsT = go_sb[:, e * QSLICE + qt * 128:
                                         e * QSLICE + (qt + 1) * 128]
                            rhs = wo_sb[:, e * E + nh * 512:e * E + (nh + 1) * 512]
                            nc.tensor.matmul(pf[:], lhsT, rhs,
                                             start=(e == 0), stop=(e == NE - 1))
                        ot = fevac.tile([128, 512], F32, tag="ot")
                        nc.vector.tensor_tensor(
                            ot[:], pf[:], bo_sb[:, nh * 512:(nh + 1) * 512],
                            AL.add)
                        nc.sync.dma_start(
                            out[qt * 128:(qt + 1) * 128,
                                nh * 512:(nh + 1) * 512], ot[:])
    nc.compile()
    return nc


def prep_in_maps(query, Wq, bq, Wk, bk, Wv, bv, Wo, bo, conv_w, conv_b):
    """Host-side layout prep. conv_b is dropped: softmax(x+c) == softmax(x)."""
    del conv_b
    scale = SQ / np.sqrt(DH)      # 1/sqrt(Dh) and the fp8 Q scale, folded
    qT = np.ascontiguousarray(query.reshape(BS, E).T)           # [E, BS]
    qTh = qT.astype(ml_dtypes.bfloat16)

    def pack_w(WT):  # [E_in, E_out] -> [128, NE*NE*128], stripe et is
        # [128, NE*128] with col (j*128+e) = WT[j*128+p, et*128+e]
        Wr = WT.reshape(NE, 128, NE, 128)          # [j, p, et, e]
        return np.ascontiguousarray(
            Wr.transpose(1, 2, 0, 3).reshape(128, NE * NE * 128))

    wq_p = pack_w((Wq.T * scale).astype(np.float32)).astype(ml_dtypes.bfloat16)
    wk_p = pack_w(Wk.T.astype(np.float32)).astype(ml_dtypes.bfloat16)
    # wo packed: stripe e is [128, E] with col eo = Wo.T[e*128+p, eo]
    wo_p = np.ascontiguousarray(
        Wo.T.reshape(NE, 128, E).transpose(1, 0, 2).reshape(128, NE * E)
    ).astype(ml_dtypes.bfloat16)
    bq_a = np.ascontiguousarray(
        (bq * scale).reshape(NE, 128).T).astype(np.float32)
    bk_a = np.ascontiguousarray(bk.reshape(NE, 128).T).astype(np.float32)
    bo_a = np.tile(bo.astype(np.float32)[None, :], (128, 1))

    in_maps = []
    for c in range(N_CORES):
        heads = [HPC * c + h for h in range(HPC)]
        # Wv^T slice packed: [128, NE*HPC*DH], col block j -> Wv[e_g, j*128+p]
        wv_cols = np.concatenate(
            [Wv[ho * DH:(ho + 1) * DH, :] for ho in heads], axis=0)  # [128,E]
        wv_p = np.ascontiguousarray(
            wv_cols.T.reshape(NE, 128, HPC * DH).transpose(1, 0, 2)
            .reshape(128, NE * HPC * DH)).astype(ml_dtypes.bfloat16)
        bv_a = np.zeros((128, HPC), np.float32)
        for h, ho in enumerate(heads):
            bv_a[0:DH, h] = bv[ho * DH:(ho + 1) * DH]
        wvec = np.zeros((128, HPC * KS * NE), np.float32)
        for h, ho in enumerate(heads):
            for dk in range(KS):
                col_v = np.repeat(conv_w[ho, :, dk] * SK, DH)  # [E]
                for cc in range(NE):
                    wvec[:, (h * KS + dk) * NE + cc] = col_v[cc * 128:(cc + 1) * 128]
        in_maps.append({
            "qTh": qTh, "wq_p": wq_p, "wk_p": wk_p, "wv_p": wv_p,
            "wo_p": wo_p, "bq": bq_a, "bk": bk_a, "bv": bv_a,
            "bo": bo_a, "wvec": wvec,
        })
    return in_maps


_NC_CACHE = {}


def kernel(**inputs) -> np.ndarray:
    in_maps = prep_in_maps(**{k: np.asarray(v) for k, v in inputs.items()})
    if "nc" not in _NC_CACHE:
        _NC_CACHE["nc"] = build_nc()
    nc = _NC_CACHE["nc"]
    res = run_bass_kernel_spmd(nc, in_maps, list(range(N_CORES)))
    full = np.concatenate([res.results[c]["out"] for c in range(N_CORES)],
                          axis=0)
    return full.reshape(B, S, E).astype(np.float32)


# revision 4
# speedup vs baseline: 126.8181x; 1.1816x over previous
"""Trainium2 Bass kernel for ConvolutionalAttention (B=2,S=2048,E=1024,H=16,KS=3).

Reference:  Q,K,V = query @ W.T + b;  scores = QK^T/sqrt(Dh) per head;
cross-head conv1d (H->H channels, kernel 3) along the key axis; softmax over
keys; out = (weights @ V) merged heads @ Wo.T + bo.

Strategy (8 cores, head-parallel, conv folded into K):
  K_conv[ho][k,(hi,d)] = sum_dk conv_w[ho,hi,dk] * K[k+dk-1,(hi,d)]
  => scores_conv[ho] = Q_full @ K_conv[ho]^T   (E=1024-deep matmul, computed
  transposed as [k,q]).  Each core owns H/8 = 2 output heads for all (b,q).

  All heavy matmuls run in fp8e4 with DoubleRow perf mode (256-deep per
  instruction).  Scales: query^T x16, Wq^T x(SQ/sqrt(Dh)), Wk^T x64; the
  products are rescaled on the PSUM evacuations and the final softmax Exp
  (scale 1/(SQ*SK)).  Pipeline per core:
    1. one pass over host-transposed fp8 query^T computes Q^T (->DRAM fp8,
       x32), K^T (->SBUF bf16, zero-padded edge cols) and V[s,d] (->SBUF
       bf16), sharing every loaded rhs tile between the three projections;
       the cross-head conv (3 shifted, per-partition-scaled taps on VectorE,
       x256) for e-chunk c is emitted right after its projections and lands
       directly in a persistent SBUF slab (no DRAM round trip);
    2. per (b, head): QK_conv fp8 DoubleRow matmuls -> PSUM f32 -> Exp on
       ScalarE (bf16 out) -> PV matmuls against ones-augmented V so the
       softmax denominator lands in PSUM row 64 -> reciprocal -> K=1-matmul
       broadcast -> normalize (+bv).  bv is exact post-softmax (weights sum
       to 1); conv_b cancels inside softmax;
    3. AllToAll (bf16) reshards (head-slice -> q-slice); final Wo projection
       of this core's 512 output rows (bf16 matmuls, f32 bias+output).
"""
import numpy as np
import ml_dtypes

import concourse.bacc as bacc
import concourse.mybir as mybir
import concourse.tile as tile
from concourse.bass_utils import run_bass_kernel_spmd

B, S, E, H, KS = 2, 2048, 1024, 16, 3
DH = E // H                  # 64
N_CORES = 8
HPC = H // N_CORES           # 2 heads per core
BS = B * S                   # 4096
QSLICE = BS // N_CORES       # 512 output rows per core
NE = E // 128                # 8 contraction chunks
NCC = NE // 2                # 4 DoubleRow (256-deep) chunks
NSC = BS // 512              # 8 s-chunks in projection pass
NKT = S // 128               # 16 k-tiles per batch
NQC = S // 512               # 4 q-chunks per batch
VROW = DH + 1                # 65: head block in augmented V
KT_PAD_W = 2 * S + 4         # [z | b0:S | z z | b1:S | z]
_B_OFF = (1, S + 3)
_PAD_COLS = (0, S + 1, S + 2, 2 * S + 3)

SIN = 16.0                   # fp8 scale on query^T input
SWK = 64.0                   # fp8 scale on Wk^T
SQ = 256.0                   # fp8 scale on Q^T output (folded with 1/sqrt(Dh))
SK = 256.0                   # fp8 scale on K_conv
EXP_SCALE = 1.0 / (SQ * SK)

F32 = mybir.dt.float32
BF16 = mybir.dt.bfloat16
F8 = mybir.dt.float8e4
F8NP = ml_dtypes.float8_e4m3
AL = mybir.AluOpType
AF = mybir.ActivationFunctionType
DR = mybir.MatmulPerfMode.DoubleRow


def build_nc(n_cores=N_CORES, collective=True):
    nc = bacc.Bacc("TRN2", target_bir_lowering=False, debug=False,
                   num_devices=n_cores)
    # inputs (host-prepped layouts; see prep_in_maps)
    qTh = nc.dram_tensor("qTh", [E, BS], F8, kind="ExternalInput")
    wq_p = nc.dram_tensor("wq_p", [128, NE * NE * 128], F8, kind="ExternalInput")
    wk_p = nc.dram_tensor("wk_p", [128, NE * NE * 128], F8, kind="ExternalInput")
    wv_p = nc.dram_tensor("wv_p", [128, NE * HPC * DH], BF16, kind="ExternalInput")
    wo_p = nc.dram_tensor("wo_p", [128, NE * E], BF16, kind="ExternalInput")
    bq = nc.dram_tensor("bq", [128, NE], F32, kind="ExternalInput")
    bk = nc.dram_tensor("bk", [128, NE], F32, kind="ExternalInput")
    bv = nc.dram_tensor("bv", [128, HPC], F32, kind="ExternalInput")
    bo = nc.dram_tensor("bo", [128, E], F32, kind="ExternalInput")
    wvec = nc.dram_tensor("wvec", [128, HPC * KS * NE], F32, kind="ExternalInput")
    out = nc.dram_tensor("out", [QSLICE, E], F32, kind="ExternalOutput")

    with tile.TileContext(nc) as tc:
        with (
            tc.tile_pool(name="dram", bufs=1, space="DRAM") as dram,
            tc.tile_pool(name="persist", bufs=1) as persist,
        ):
            qproj_dram = dram.tile([E, BS], F8)
            a2a_in = dram.tile([N_CORES * 128, QSLICE], BF16)
            a2a_out = dram.tile([N_CORES * 128, QSLICE], BF16)

            # K_conv stays resident in SBUF: [p, h, cc, i, b, k] where the
            # conv rows for e = cc*256 + i*128 + p live at pair-slot i (the
            # DoubleRow weight layout, i-stride B*S bytes).
            kconv_sb = persist.tile([128, HPC, NCC, 2, B, S], F8)
            # augmented V: cols = g*(HPC*VROW) + h*VROW + [0..63]=d, 64=ones
            # where g = b*NKT + kt is the global k-tile index (32 of them)
            v_sb = persist.tile([128, B * NKT * HPC * VROW], BF16)
            bv_sb = persist.tile([128, HPC], F32)
            wvec_sb = persist.tile([128, HPC * KS * NE], F32)
            ones_sb = persist.tile([1, DH], BF16)
            nc.sync.dma_start(bv_sb[:], bv[:, :])
            nc.sync.dma_start(wvec_sb[:], wvec[:, :])
            nc.vector.memset(ones_sb[:], 1.0)
            for g in range(B * NKT):
                for h in range(HPC):
                    c0 = g * HPC * VROW + h * VROW + DH
                    nc.vector.memset(v_sb[:, c0:c0 + 1], 1.0)

            # ------- phase 1: projections (+ K_conv interleaved) -------
            with (
                tc.tile_pool(name="proj", bufs=1) as proj,
                tc.tile_pool(name="pw", bufs=2) as pw,
                tc.tile_pool(name="pevac", bufs=3) as pevac,
                tc.tile_pool(name="kcv", bufs=2) as kcv,
                tc.tile_pool(name="ppsum", bufs=2, space="PSUM") as ppsum,
                tc.tile_pool(name="vpsum", bufs=4, space="PSUM") as vpsum,
            ):
                qt_full = proj.tile([128, NE, BS], F8, tag="qtfull")
                kt_pad = proj.tile([128, NE * KT_PAD_W], BF16, tag="ktpad")
                wv_sb = proj.tile([128, NE * HPC * DH], BF16, tag="wv")
                bq_sb = proj.tile([128, NE], F32, tag="bq")
                bk_sb = proj.tile([128, NE], F32, tag="bk")
                for j in range(NE):
                    nc.sync.dma_start(qt_full[:, j, :],
                                      qTh[j * 128:(j + 1) * 128, :])
                nc.sync.dma_start(wv_sb[:], wv_p[:, :])
                nc.sync.dma_start(bq_sb[:], bq[:, :])
                nc.sync.dma_start(bk_sb[:], bk[:, :])
                for c in range(NE):
                    for pc in _PAD_COLS:
                        col = c * KT_PAD_W + pc
                        nc.vector.memset(kt_pad[:, col:col + 1], 0.0)

                def emit_kconv(c):
                    # cross-head conv along k on VectorE for e-chunk c;
                    # writes land directly in the persistent fp8 slab
                    cc, i = divmod(c, 2)
                    for h in range(HPC):
                        for b_i in range(B):
                            base = c * KT_PAD_W + _B_OFF[b_i] - 1
                            t0 = kcv.tile([128, S], BF16, tag="t0")
                            t1 = kcv.tile([128, S], BF16, tag="t1")

                            def wv_(dk):
                                col = (h * KS + dk) * NE + c
                                return wvec_sb[:, col:col + 1]

                            nc.vector.tensor_scalar(
                                t0[:], kt_pad[:, base:base + S],
                                wv_(0), None, AL.mult)
                            nc.vector.tensor_scalar(
                                t1[:], kt_pad[:, base + 1:base + 1 + S],
                                wv_(1), None, AL.mult)
                            nc.vector.tensor_tensor(t0[:], t0[:], t1[:], AL.add)
                            nc.vector.tensor_scalar(
                                t1[:], kt_pad[:, base + 2:base + 2 + S],
                                wv_(2), None, AL.mult)
                            nc.vector.tensor_tensor(
                                kconv_sb[:, h, cc, i, b_i, :],
                                t0[:], t1[:], AL.add)

                # Q^T and K^T: for each e-tile stream the packed weight stripe
                for et in range(NE):
                    wq_sb = pw.tile([128, NE, 128], F8, tag="wqs")
                    wk_sb = pw.tile([128, NE, 128], F8, tag="wks")
                    nc.sync.dma_start(wq_sb[:], wq_p[:, et * E:(et + 1) * E])
                    nc.sync.dma_start(wk_sb[:], wk_p[:, et * E:(et + 1) * E])
                    for sc in range(NSC):
                        b_i, sc_i = divmod(sc, NQC)
                        pq = ppsum.tile([128, 512], F32, tag="pq")
                        pk = ppsum.tile([128, 512], F32, tag="pk")
                        for cc in range(NCC):
                            rhs = qt_full[:, 2 * cc:2 * cc + 2,
                                          sc * 512:(sc + 1) * 512]
                            nc.tensor.matmul(pq[:],
                                             wq_sb[:, 2 * cc:2 * cc + 2, :],
                                             rhs, start=(cc == 0),
                                             stop=(cc == NCC - 1), perf_mode=DR)
                        for cc in range(NCC):
                            rhs = qt_full[:, 2 * cc:2 * cc + 2,
                                          sc * 512:(sc + 1) * 512]
                            nc.tensor.matmul(pk[:],
                                             wk_sb[:, 2 * cc:2 * cc + 2, :],
                                             rhs, start=(cc == 0),
                                             stop=(cc == NCC - 1), perf_mode=DR)
                        qe = pevac.tile([128, 512], F8, tag="qevac")
                        nc.scalar.activation(qe[:], pq[:], AF.Identity,
                                             bias=bq_sb[:, et:et + 1],
                                             scale=1.0 / SIN)
                        nc.sync.dma_start(
                            qproj_dram[et * 128:(et + 1) * 128,
                                       sc * 512:(sc + 1) * 512], qe[:])
                        kcol = et * KT_PAD_W + _B_OFF[b_i] + sc_i * 512
                        nc.scalar.activation(kt_pad[:, kcol:kcol + 512], pk[:],
                                             AF.Identity,
                                             bias=bk_sb[:, et:et + 1],
                                             scale=1.0 / (SIN * SWK))
                    emit_kconv(et)

                # V: lhsT = raw query^T tiles (fp8), rhs = packed Wv^T slice
                for g in range(B * NKT):           # g = s-tile = global k-tile
                    pv = vpsum.tile([128, HPC * DH], F32, tag="pv")
                    for j in range(NE):
                        lhsT = qt_full[:, j, g * 128:(g + 1) * 128]
                        nc.tensor.matmul(
                            pv[:], lhsT,
                            wv_sb[:, j * HPC * DH:(j + 1) * HPC * DH],
                            start=(j == 0), stop=(j == NE - 1))
                    for h in range(HPC):
                        c0 = g * HPC * VROW + h * VROW
                        nc.scalar.activation(v_sb[:, c0:c0 + DH],
                                             pv[:, h * DH:(h + 1) * DH],
                                             AF.Identity, scale=1.0 / SIN)

            # ---------------- phase 2: attention ----------------
            with (
                tc.tile_pool(name="attn", bufs=1) as attn,
                tc.tile_pool(name="esb", bufs=6) as esb,
                tc.tile_pool(name="norm", bufs=4) as norm,
                tc.tile_pool(name="qkpsum", bufs=2, space="PSUM") as qkpsum,
                tc.tile_pool(name="pvpsum", bufs=4, space="PSUM") as pvpsum,
                tc.tile_pool(name="ivpsum", bufs=2, space="PSUM") as ivpsum,
            ):
                for b_i in range(B):
                    qp_res = attn.tile([128, NE, S], F8, tag="qpres")
                    for j in range(NE):
                        nc.sync.dma_start(
                            qp_res[:, j, :],
                            qproj_dram[j * 128:(j + 1) * 128,
                                       b_i * S:(b_i + 1) * S])
                    for h in range(HPC):
                        pvs = [pvpsum.tile([VROW, 512], F32, tag="pvp",
                                           name=f"pv{qq}")
                               for qq in range(NQC)]
                        inv_sb = norm.tile([1, S], BF16, tag="inv")
                        for kt in range(NKT):
                            g = b_i * NKT + kt
                            for qc in range(NQC):
                                ps = qkpsum.tile([128, 512], F32, tag="qk")
                                for cc in range(NCC):
                                    nc.tensor.matmul(
                                        ps[:],
                                        kconv_sb[:, h, cc, :, b_i,
                                                 kt * 128:(kt + 1) * 128],
                                        qp_res[:, 2 * cc:2 * cc + 2,
                                               qc * 512:(qc + 1) * 512],
                                        start=(cc == 0), stop=(cc == NCC - 1),
                                        perf_mode=DR)
                                ex = esb.tile([128, 512], BF16, tag="exp")
                                nc.scalar.activation(ex[:], ps[:], AF.Exp,
                                                     scale=EXP_SCALE)
                                c0 = g * HPC * VROW + h * VROW
                                nc.tensor.matmul(
                                    pvs[qc][:], v_sb[:, c0:c0 + VROW],
                                    ex[:], start=(kt == 0),
                                    stop=(kt == NKT - 1))
                        # normalize + bias, ship to a2a bounce
                        for qc in range(NQC):
                            with nc.allow_low_precision(
                                    reason="softmax denom bf16 bcast"):
                                nc.vector.reciprocal(
                                    inv_sb[0:1, qc * 512:(qc + 1) * 512],
                                    pvs[qc][DH:DH + 1, :])
                            pi = ivpsum.tile([DH, 512], F32, tag="iv")
                            nc.tensor.matmul(
                                pi[:], ones_sb[0:1, :],
                                inv_sb[0:1, qc * 512:(qc + 1) * 512],
                                start=True, stop=True)
                            ib = norm.tile([DH, 512], F32, tag="invbc")
                            nc.scalar.activation(ib[:], pi[:], AF.Copy)
                            ho = norm.tile([DH, 512], F32, tag="ho")
                            ho16 = norm.tile([DH, 512], BF16, tag="ho16")
                            nc.vector.tensor_tensor(ho[:], pvs[qc][0:DH, :],
                                                    ib[:], AL.mult)
                            nc.vector.tensor_scalar(
                                ho16[:], ho[:], bv_sb[0:DH, h:h + 1], None,
                                AL.add)
                            piece = b_i * NQC + qc
                            r0 = piece * 128 + h * DH
                            nc.sync.dma_start(a2a_in[r0:r0 + DH, :], ho16[:])

            # ---------------- phase 3: exchange + output proj ----------------
            if collective:
                nc.gpsimd.collective_compute(
                    "AllToAll", AL.bypass,
                    replica_groups=[list(range(n_cores))],
                    ins=[a2a_in.opt()], outs=[a2a_out.opt()])
            else:
                nc.sync.dma_start(a2a_out[:, :], a2a_in[:, :])

            with (
                tc.tile_pool(name="fin", bufs=1) as fin,
                tc.tile_pool(name="fevac", bufs=3) as fevac,
                tc.tile_pool(name="fpsum", bufs=2, space="PSUM") as fpsum,
            ):
                go_sb = fin.tile([128, NE * QSLICE], BF16, tag="go")
                wo_sb = fin.tile([128, NE * E], BF16, tag="wo")
                bo_sb = fin.tile([128, E], F32, tag="bo")
                for e in range(NE):
                    nc.sync.dma_start(go_sb[:, e * QSLICE:(e + 1) * QSLICE],
                                      a2a_out[e * 128:(e + 1) * 128, :])
                nc.sync.dma_start(wo_sb[:], wo_p[:, :])
                nc.sync.dma_start(bo_sb[:], bo[:, :])
                for qt in range(QSLICE // 128):
                    for nh in range(E // 512):
                        pf = fpsum.tile([128, 512], F32, tag="pf")
                        for e in range(NE):
                            lhsT = go_sb[:, e * QSLICE + qt * 128:
                                         e * QSLICE + (qt + 1) * 128]
                            rhs = wo_sb[:, e * E + nh * 512:e * E + (nh + 1) * 512]
                            nc.tensor.matmul(pf[:], lhsT, rhs,
                                             start=(e == 0), stop=(e == NE - 1))
                        ot = fevac.tile([128, 512], F32, tag="ot")
                        nc.vector.tensor_tensor(
                            ot[:], pf[:], bo_sb[:, nh * 512:(nh + 1) * 512],
                            AL.add)
                        nc.sync.dma_start(
                            out[qt * 128:(qt + 1) * 128,
                                nh * 512:(nh + 1) * 512], ot[:])
    nc.compile()
    return nc


def prep_in_maps(query, Wq, bq, Wk, bk, Wv, bv, Wo, bo, conv_w, conv_b):
    """Host-side layout prep. conv_b is dropped: softmax(x+c) == softmax(x)."""
    del conv_b
    scale = SQ / np.sqrt(DH)      # 1/sqrt(Dh) and the fp8 Q scale, folded
    qT = np.ascontiguousarray(query.reshape(BS, E).T)           # [E, BS]
    qTh = (qT * SIN).astype(F8NP)

    def pack_w(WT):  # [E_in, E_out] -> [128, NE*NE*128], stripe et is
        # [128, NE*128] with col (j*128+e) = WT[j*128+p, et*128+e]
        Wr = WT.reshape(NE, 128, NE, 128)          # [j, p, et, e]
        return np.ascontiguousarray(
            Wr.transpose(1, 2, 0, 3).reshape(128, NE * NE * 128))

    wq_p = pack_w((Wq.T * scale).astype(np.float32)).astype(F8NP)
    wk_p = pack_w((Wk.T * SWK).astype(np.float32)).astype(F8NP)
    # wo packed: stripe e is [128, E] with col eo = Wo.T[e*128+p, eo]
    wo_p = np.ascontiguousarray(
        Wo.T.reshape(NE, 128, E).transpose(1, 0, 2).reshape(128, NE * E)
    ).astype(ml_dtypes.bfloat16)
    bq_a = np.ascontiguousarray(
        (bq * scale).reshape(NE, 128).T).astype(np.float32)
    bk_a = np.ascontiguousarray(bk.reshape(NE, 128).T).astype(np.float32)
    bo_a = np.tile(bo.astype(np.float32)[None, :], (128, 1))

    in_maps = []
    for c in range(N_CORES):
        heads = [HPC * c + h for h in range(HPC)]
        # Wv^T slice packed: [128, NE*HPC*DH], col block j -> Wv[e_g, j*128+p]
        wv_cols = np.concatenate(
            [Wv[ho * DH:(ho + 1) * DH, :] for ho in heads], axis=0)  # [128,E]
        wv_p = np.ascontiguousarray(
            wv_cols.T.reshape(NE, 128, HPC * DH).transpose(1, 0, 2)
            .reshape(128, NE * HPC * DH)).astype(ml_dtypes.bfloat16)
        bv_a = np.zeros((128, HPC), np.float32)
        for h, ho in enumerate(heads):
            bv_a[0:DH, h] = bv[ho * DH:(ho + 1) * DH]
        wvec = np.zeros((128, HPC * KS * NE), np.float32)
        for h, ho in enumerate(heads):
            for dk in range(KS):
                col_v = np.repeat(conv_w[ho, :, dk] * SK, DH)  # [E]
                for cc in range(NE):
                    wvec[:, (h * KS + dk) * NE + cc] = col_v[cc * 128:(cc + 1) * 128]
        in_maps.append({
            "qTh": qTh, "wq_p": wq_p, "wk_p": wk_p, "wv_p": wv_p,
            "wo_p": wo_p, "bq": bq_a, "bk": bk_a, "bv": bv_a,
            "bo": bo_a, "wvec": wvec,
        })
    return in_maps


_NC_CACHE = {}


def kernel(**inputs) -> np.ndarray:
    in_maps = prep_in_maps(**{k: np.asarray(v) for k, v in inputs.items()})
    if "nc" not in _NC_CACHE:
        _NC_CACHE["nc"] = build_nc()
    nc = _NC_CACHE["nc"]
    res = run_bass_kernel_spmd(nc, in_maps, list(range(N_CORES)))
    full = np.concatenate([res.results[c]["out"] for c in range(N_CORES)],
                          axis=0)
    return full.reshape(B, S, E).astype(np.float32)


# revision 7
# speedup vs baseline: 128.9727x; 1.0170x over previous
"""Trainium2 Bass kernel for ConvolutionalAttention (B=2,S=2048,E=1024,H=16,KS=3).

Reference:  Q,K,V = query @ W.T + b;  scores = QK^T/sqrt(Dh) per head;
cross-head conv1d (H->H channels, kernel 3) along the key axis; softmax over
keys; out = (weights @ V) merged heads @ Wo.T + bo.

Strategy (8 cores, head-parallel, conv folded into K):
  K_conv[ho][k,(hi,d)] = sum_dk conv_w[ho,hi,dk] * K[k+dk-1,(hi,d)]
  => scores_conv[ho] = Q_full @ K_conv[ho]^T   (E=1024-deep matmul, computed
  transposed as [k,q]).  Each core owns H/8 = 2 output heads for all (b,q).

  All heavy matmuls run in fp8e4 with DoubleRow perf mode (256-deep per
  instruction).  Scales: query^T x16, Wq^T x(SQ/sqrt(Dh)), Wk^T x64; the
  products are rescaled on the PSUM evacuations and the final softmax Exp
  (scale 1/(SQ*SK)).  Pipeline per core:
    1. one pass over host-transposed fp8 query^T computes Q^T (->DRAM fp8,
       x32), K^T (->SBUF bf16, zero-padded edge cols) and V[s,d] (->SBUF
       bf16), sharing every loaded rhs tile between the three projections;
       the cross-head conv (3 shifted, per-partition-scaled taps on VectorE,
       x256) for e-chunk c is emitted right after its projections and lands
       directly in a persistent SBUF slab (no DRAM round trip);
    2. per (b, head): QK_conv fp8 DoubleRow matmuls -> PSUM f32 -> Exp on
       ScalarE (bf16 out) -> PV matmuls against ones-augmented V so the
       softmax denominator lands in PSUM row 64 -> reciprocal -> K=1-matmul
       broadcast -> normalize (+bv).  bv is exact post-softmax (weights sum
       to 1); conv_b cancels inside softmax;
    3. AllToAll (bf16) reshards (head-slice -> q-slice); final Wo projection
       of this core's 512 output rows (bf16 matmuls, f32 bias+output).
"""
import numpy as np
import ml_dtypes

import concourse.bacc as bacc
import concourse.mybir as mybir
import concourse.tile as tile
from concourse.bass_utils import run_bass_kernel_spmd

B, S, E, H, KS = 2, 2048, 1024, 16, 3
DH = E // H                  # 64
N_CORES = 8
HPC = H // N_CORES           # 2 heads per core
BS = B * S                   # 4096
QSLICE = BS // N_CORES       # 512 output rows per core
NE = E // 128                # 8 contraction chunks
NCC = NE // 2                # 4 DoubleRow (256-deep) chunks
NSC = BS // 512              # 8 s-chunks in projection pass
NKT = S // 128               # 16 k-tiles per batch
NQC = S // 512               # 4 q-chunks per batch
VROW = DH + 1                # 65: head block in augmented V
KT_PAD_W = 2 * S + 4         # [z | b0:S | z z | b1:S | z]
_B_OFF = (1, S + 3)
_PAD_COLS = (0, S + 1, S + 2, 2 * S + 3)

SIN = 16.0                   # fp8 scale on query^T input
SWK = 64.0                   # fp8 scale on Wk^T
SQ = 256.0                   # fp8 scale on Q^T output (folded with 1/sqrt(Dh))
SK = 256.0                   # fp8 scale on K_conv
EXP_SCALE = 1.0 / (SQ * SK)

F32 = mybir.dt.float32
BF16 = mybir.dt.bfloat16
F8 = mybir.dt.float8e4
F8NP = ml_dtypes.float8_e4m3
AL = mybir.AluOpType
AF = mybir.ActivationFunctionType
DR = mybir.MatmulPerfMode.DoubleRow


def build_nc(n_cores=N_CORES, collective=True):
    nc = bacc.Bacc("TRN2", target_bir_lowering=False, debug=False,
                   num_devices=n_cores)
    # inputs (host-prepped layouts; see prep_in_maps)
    qTh = nc.dram_tensor("qTh", [E, BS], F8, kind="ExternalInput")
    wq_p = nc.dram_tensor("wq_p", [128, NE * NE * 128], F8, kind="ExternalInput")
    wk_p = nc.dram_tensor("wk_p", [128, NE * NE * 128], F8, kind="ExternalInput")
    wv_p = nc.dram_tensor("wv_p", [128, NE * HPC * DH], BF16, kind="ExternalInput")
    wo_p = nc.dram_tensor("wo_p", [128, NE * E], BF16, kind="ExternalInput")
    bq = nc.dram_tensor("bq", [128, NE], F32, kind="ExternalInput")
    bk = nc.dram_tensor("bk", [128, NE], F32, kind="ExternalInput")
    bv = nc.dram_tensor("bv", [128, HPC], F32, kind="ExternalInput")
    bo = nc.dram_tensor("bo", [128, E], F32, kind="ExternalInput")
    wvec = nc.dram_tensor("wvec", [128, HPC * KS * NE], F32, kind="ExternalInput")
    out = nc.dram_tensor("out", [QSLICE, E], F32, kind="ExternalOutput")

    with tile.TileContext(nc) as tc:
        with (
            tc.tile_pool(name="dram", bufs=1, space="DRAM") as dram,
            tc.tile_pool(name="persist", bufs=1) as persist,
        ):
            qproj_dram = dram.tile([E, BS], F8)
            a2a_in = dram.tile([N_CORES * 128, QSLICE], BF16)
            a2a_out = dram.tile([N_CORES * 128, QSLICE], BF16)

            # K_conv stays resident in SBUF: [p, h, cc, i, b, k] where the
            # conv rows for e = cc*256 + i*128 + p live at pair-slot i (the
            # DoubleRow weight layout, i-stride B*S bytes).
            kconv_sb = persist.tile([128, HPC, NCC, 2, B, S], F8)
            # augmented V: cols = g*(HPC*VROW) + h*VROW + [0..63]=d, 64=ones
            # where g = b*NKT + kt is the global k-tile index (32 of them)
            v_sb = persist.tile([128, B * NKT * HPC * VROW], BF16)
            bv_sb = persist.tile([128, HPC], F32)
            wvec_sb = persist.tile([128, HPC * KS * NE], F32)
            ones_sb = persist.tile([1, DH], BF16)
            nc.sync.dma_start(bv_sb[:], bv[:, :])
            nc.sync.dma_start(wvec_sb[:], wvec[:, :])
            nc.vector.memset(ones_sb[:], 1.0)
            for g in range(B * NKT):
                for h in range(HPC):
                    c0 = g * HPC * VROW + h * VROW + DH
                    nc.vector.memset(v_sb[:, c0:c0 + 1], 1.0)

            # ------- phase 1: projections (+ K_conv interleaved) -------
            with (
                tc.tile_pool(name="proj", bufs=1) as proj,
                tc.tile_pool(name="pw", bufs=2) as pw,
                tc.tile_pool(name="pevac", bufs=3) as pevac,
                tc.tile_pool(name="kcv", bufs=2) as kcv,
                tc.tile_pool(name="ppsum", bufs=2, space="PSUM") as ppsum,
                tc.tile_pool(name="vpsum", bufs=4, space="PSUM") as vpsum,
            ):
                qt_full = proj.tile([128, NE, BS], F8, tag="qtfull")
                kt_pad = proj.tile([128, NE * KT_PAD_W], BF16, tag="ktpad")
                wv_sb = proj.tile([128, NE * HPC * DH], BF16, tag="wv")
                bq_sb = proj.tile([128, NE], F32, tag="bq")
                bk_sb = proj.tile([128, NE], F32, tag="bk")
                for j in range(NE):
                    nc.sync.dma_start(qt_full[:, j, :],
                                      qTh[j * 128:(j + 1) * 128, :])
                nc.sync.dma_start(wv_sb[:], wv_p[:, :])
                nc.sync.dma_start(bq_sb[:], bq[:, :])
                nc.sync.dma_start(bk_sb[:], bk[:, :])
                for c in range(NE):
                    for pc in _PAD_COLS:
                        col = c * KT_PAD_W + pc
                        nc.vector.memset(kt_pad[:, col:col + 1], 0.0)

                def emit_kconv(c):
                    # cross-head conv along k on VectorE for e-chunk c;
                    # writes land directly in the persistent fp8 slab.
                    # b-major so batch 0 (attention starts there) drains first
                    cc, i = divmod(c, 2)
                    for b_i in range(B):
                        for h in range(HPC):
                            base = c * KT_PAD_W + _B_OFF[b_i] - 1
                            t0 = kcv.tile([128, S], BF16, tag="t0")
                            t1 = kcv.tile([128, S], BF16, tag="t1")

                            def wv_(dk):
                                col = (h * KS + dk) * NE + c
                                return wvec_sb[:, col:col + 1]

                            nc.vector.tensor_scalar(
                                t0[:], kt_pad[:, base:base + S],
                                wv_(0), None, AL.mult)
                            nc.vector.tensor_scalar(
                                t1[:], kt_pad[:, base + 1:base + 1 + S],
                                wv_(1), None, AL.mult)
                            nc.vector.tensor_tensor(t0[:], t0[:], t1[:], AL.add)
                            nc.vector.tensor_scalar(
                                t1[:], kt_pad[:, base + 2:base + 2 + S],
                                wv_(2), None, AL.mult)
                            nc.vector.tensor_tensor(
                                kconv_sb[:, h, cc, i, b_i, :],
                                t0[:], t1[:], AL.add)

                # Q^T and K^T: for each e-tile stream the packed weight stripe
                for et in range(NE):
                    wq_sb = pw.tile([128, NE, 128], F8, tag="wqs")
                    wk_sb = pw.tile([128, NE, 128], F8, tag="wks")
                    nc.sync.dma_start(wq_sb[:], wq_p[:, et * E:(et + 1) * E])
                    nc.sync.dma_start(wk_sb[:], wk_p[:, et * E:(et + 1) * E])
                    for sc in range(NSC):
                        b_i, sc_i = divmod(sc, NQC)
                        pq = ppsum.tile([128, 512], F32, tag="pq")
                        pk = ppsum.tile([128, 512], F32, tag="pk")
                        for cc in range(NCC):
                            rhs = qt_full[:, 2 * cc:2 * cc + 2,
                                          sc * 512:(sc + 1) * 512]
                            nc.tensor.matmul(pq[:],
                                             wq_sb[:, 2 * cc:2 * cc + 2, :],
                                             rhs, start=(cc == 0),
                                             stop=(cc == NCC - 1), perf_mode=DR)
                        for cc in range(NCC):
                            rhs = qt_full[:, 2 * cc:2 * cc + 2,
                                          sc * 512:(sc + 1) * 512]
                            nc.tensor.matmul(pk[:],
                                             wk_sb[:, 2 * cc:2 * cc + 2, :],
                                             rhs, start=(cc == 0),
                                             stop=(cc == NCC - 1), perf_mode=DR)
                        qe = pevac.tile([128, 512], F8, tag="qevac")
                        nc.scalar.activation(qe[:], pq[:], AF.Identity,
                                             bias=bq_sb[:, et:et + 1],
                                             scale=1.0 / SIN)
                        nc.sync.dma_start(
                            qproj_dram[et * 128:(et + 1) * 128,
                                       sc * 512:(sc + 1) * 512], qe[:])
                        kcol = et * KT_PAD_W + _B_OFF[b_i] + sc_i * 512
                        nc.scalar.activation(kt_pad[:, kcol:kcol + 512], pk[:],
                                             AF.Identity,
                                             bias=bk_sb[:, et:et + 1],
                                             scale=1.0 / (SIN * SWK))
                    emit_kconv(et)

                # V: lhsT = raw query^T tiles (fp8), rhs = packed Wv^T slice
                for g in range(B * NKT):           # g = s-tile = global k-tile
                    pv = vpsum.tile([128, HPC * DH], F32, tag="pv")
                    for j in range(NE):
                        lhsT = qt_full[:, j, g * 128:(g + 1) * 128]
                        nc.tensor.matmul(
                            pv[:], lhsT,
                            wv_sb[:, j * HPC * DH:(j + 1) * HPC * DH],
                            start=(j == 0), stop=(j == NE - 1))
                    for h in range(HPC):
                        c0 = g * HPC * VROW + h * VROW
                        nc.scalar.activation(v_sb[:, c0:c0 + DH],
                                             pv[:, h * DH:(h + 1) * DH],
                                             AF.Identity, scale=1.0 / SIN)

            # ---------------- phase 2: attention ----------------
            with (
                tc.tile_pool(name="attn", bufs=1) as attn,
                tc.tile_pool(name="esb", bufs=6) as esb,
                tc.tile_pool(name="norm", bufs=4) as norm,
                tc.tile_pool(name="qkpsum", bufs=2, space="PSUM") as qkpsum,
                tc.tile_pool(name="pvpsum", bufs=4, space="PSUM") as pvpsum,
                tc.tile_pool(name="ivpsum", bufs=2, space="PSUM") as ivpsum,
            ):
                for b_i in range(B):
                    qp_res = attn.tile([128, NE, S], F8, tag="qpres")
                    for j in range(NE):
                        nc.sync.dma_start(
                            qp_res[:, j, :],
                            qproj_dram[j * 128:(j + 1) * 128,
                                       b_i * S:(b_i + 1) * S])
                    for h in range(HPC):
                        pvs = [pvpsum.tile([VROW, 512], F32, tag="pvp",
                                           name=f"pv{qq}")
                               for qq in range(NQC)]
                        inv_sb = norm.tile([1, S], BF16, tag="inv")
                        for kt in range(NKT):
                            g = b_i * NKT + kt
                            for qc in range(NQC):
                                ps = qkpsum.tile([128, 512], F32, tag="qk")
                                for cc in range(NCC):
                                    nc.tensor.matmul(
                                        ps[:],
                                        kconv_sb[:, h, cc, :, b_i,
                                                 kt * 128:(kt + 1) * 128],
                                        qp_res[:, 2 * cc:2 * cc + 2,
                                               qc * 512:(qc + 1) * 512],
                                        start=(cc == 0), stop=(cc == NCC - 1),
                                        perf_mode=DR)
                                ex = esb.tile([128, 512], BF16, tag="exp")
                                nc.scalar.activation(ex[:], ps[:], AF.Exp,
                                                     scale=EXP_SCALE)
                                c0 = g * HPC * VROW + h * VROW
                                nc.tensor.matmul(
                                    pvs[qc][:], v_sb[:, c0:c0 + VROW],
                                    ex[:], start=(kt == 0),
                                    stop=(kt == NKT - 1))
                        # evacuate PV PSUM -> SBUF right away so the banks
                        # free for the next head's QK/PV while we normalize
                        pvses = []
                        for qc in range(NQC):
                            pvse = norm.tile([VROW, 512], F32, tag="pvse",
                                             name=f"pvse{qc}", bufs=8)
                            nc.scalar.activation(pvse[:], pvs[qc][:], AF.Copy)
                            pvses.append(pvse)
                        # normalize + bias, ship to a2a bounce
                        for qc in range(NQC):
                            with nc.allow_low_precision(
                                    reason="softmax denom bf16 bcast"):
                                nc.vector.reciprocal(
                                    inv_sb[0:1, qc * 512:(qc + 1) * 512],
                                    pvses[qc][DH:DH + 1, :])
                            pi = ivpsum.tile([DH, 512], F32, tag="iv")
                            nc.tensor.matmul(
                                pi[:], ones_sb[0:1, :],
                                inv_sb[0:1, qc * 512:(qc + 1) * 512],
                                start=True, stop=True)
                            ho = norm.tile([DH, 512], F32, tag="ho")
                            ho16 = norm.tile([DH, 512], BF16, tag="ho16")
                            nc.vector.tensor_tensor(ho[:], pvses[qc][0:DH, :],
                                                    pi[:], AL.mult)
                            nc.vector.tensor_scalar(
                                ho16[:], ho[:], bv_sb[0:DH, h:h + 1], None,
                                AL.add)
                            piece = b_i * NQC + qc
                            r0 = piece * 128 + h * DH
                            nc.sync.dma_start(a2a_in[r0:r0 + DH, :], ho16[:])

            # ---------------- phase 3: exchange + output proj ----------------
            with (
                tc.tile_pool(name="fin", bufs=1) as fin,
                tc.tile_pool(name="fevac", bufs=3) as fevac,
                tc.tile_pool(name="fpsum", bufs=2, space="PSUM") as fpsum,
            ):
                wo_sb = fin.tile([128, NE * E], BF16, tag="wo")
                bo_sb = fin.tile([128, E], F32, tag="bo")
                nc.sync.dma_start(wo_sb[:], wo_p[:, :])
                nc.sync.dma_start(bo_sb[:], bo[:, :])

                if collective:
                    nc.gpsimd.collective_compute(
                        "AllToAll", AL.bypass,
                        replica_groups=[list(range(n_cores))],
                        ins=[a2a_in.opt()], outs=[a2a_out.opt()])
                else:
                    nc.sync.dma_start(a2a_out[:, :], a2a_in[:, :])

                go_sb = fin.tile([128, NE * QSLICE], BF16, tag="go")
                for e in range(NE):
                    nc.sync.dma_start(go_sb[:, e * QSLICE:(e + 1) * QSLICE],
                                      a2a_out[e * 128:(e + 1) * 128, :])
                for qt in range(QSLICE // 128):
                    for nh in range(E // 512):
                        pf = fpsum.tile([128, 512], F32, tag="pf")
                        for e in range(NE):
                            lhsT = go_sb[:, e * QSLICE + qt * 128:
                                         e * QSLICE + (qt + 1) * 128]
                            rhs = wo_sb[:, e * E + nh * 512:e * E + (nh + 1) * 512]
                            nc.tensor.matmul(pf[:], lhsT, rhs,
                                             start=(e == 0), stop=(e == NE - 1))
                        ot = fevac.tile([128, 512], F32, tag="ot")
                        nc.vector.tensor_tensor(
                            ot[:], pf[:], bo_sb[:, nh * 512:(nh + 1) * 512],
                            AL.add)
                        nc.sync.dma_start(
                            out[qt * 128:(qt + 1) * 128,
                                nh * 512:(nh + 1) * 512], ot[:])
    nc.compile()
    return nc


def prep_in_maps(query, Wq, bq, Wk, bk, Wv, bv, Wo, bo, conv_w, conv_b):
    """Host-side layout prep. conv_b is dropped: softmax(x+c) == softmax(x)."""
    del conv_b
    scale = SQ / np.sqrt(DH)      # 1/sqrt(Dh) and the fp8 Q scale, folded
    qT = np.ascontiguousarray(query.reshape(BS, E).T)           # [E, BS]
    qTh = (qT * SIN).astype(F8NP)

    def pack_w(WT):  # [E_in, E_out] -> [128, NE*NE*128], stripe et is
        # [128, NE*128] with col (j*128+e) = WT[j*128+p, et*128+e]
        Wr = WT.reshape(NE, 128, NE, 128)          # [j, p, et, e]
        return np.ascontiguousarray(
            Wr.transpose(1, 2, 0, 3).reshape(128, NE * NE * 128))

    wq_p = pack_w((Wq.T * scale).astype(np.float32)).astype(F8NP)
    wk_p = pack_w((Wk.T * SWK).astype(np.float32)).astype(F8NP)
    # wo packed: stripe e is [128, E] with col eo = Wo.T[e*128+p, eo]
    wo_p = np.ascontiguousarray(
        Wo.T.reshape(NE, 128, E).transpose(1, 0, 2).reshape(128, NE * E)
    ).astype(ml_dtypes.bfloat16)
    bq_a = np.ascontiguousarray(
        (bq * scale).reshape(NE, 128).T).astype(np.float32)
    bk_a = np.ascontiguousarray(bk.reshape(NE, 128).T).astype(np.float32)
    bo_a = np.tile(bo.astype(np.float32)[None, :], (128, 1))

    in_maps = []
    for c in range(N_CORES):
        heads = [HPC * c + h for h in range(HPC)]
        # Wv^T slice packed: [128, NE*HPC*DH], col block j -> Wv[e_g, j*128+p]
        wv_cols = np.concatenate(
            [Wv[ho * DH:(ho + 1) * DH, :] for ho in heads], axis=0)  # [128,E]
        wv_p = np.ascontiguousarray(
            wv_cols.T.reshape(NE, 128, HPC * DH).transpose(1, 0, 2)
            .reshape(128, NE * HPC * DH)).astype(ml_dtypes.bfloat16)
        bv_a = np.zeros((128, HPC), np.float32)
        for h, ho in enumerate(heads):
            bv_a[0:DH, h] = bv[ho * DH:(ho + 1) * DH]
        wvec = np.zeros((128, HPC * KS * NE), np.float32)
        for h, ho in enumerate(heads):
            for dk in range(KS):
                col_v = np.repeat(conv_w[ho, :, dk] * SK, DH)  # [E]
                for cc in range(NE):
                    wvec[:, (h * KS + dk) * NE + cc] = col_v[cc * 128:(cc + 1) * 128]
        in_maps.append({
            "qTh": qTh, "wq_p": wq_p, "wk_p": wk_p, "wv_p": wv_p,
            "wo_p": wo_p, "bq": bq_a, "bk": bk_a, "bv": bv_a,
            "bo": bo_a, "wvec": wvec,
        })
    return in_maps


_NC_CACHE = {}


def kernel(**inputs) -> np.ndarray:
    in_maps = prep_in_maps(**{k: np.asarray(v) for k, v in inputs.items()})
    if "nc" not in _NC_CACHE:
        _NC_CACHE["nc"] = build_nc()
    nc = _NC_CACHE["nc"]
    res = run_bass_kernel_spmd(nc, in_maps, list(range(N_CORES)))
    full = np.concatenate([res.results[c]["out"] for c in range(N_CORES)],
                          axis=0)
    return full.reshape(B, S, E).astype(np.float32)
